# revision 2
# baseline (speedup 1.0000x reference)
"""Multi-head attention kernel for 8 TRN2 NeuronCores.

Problem: b=2, n=2048, d=1024, heads=16, hd=64.
  q/k/v = x @ W{q,k,v}.T (+ zero bias)
  per head: softmax(q k^T / sqrt(d)) @ v
  out = concat @ Wo.T (+ zero bias)

Sharding (8 cores): data-parallel over batch (2) x tensor-parallel over
heads (16 heads -> 4 groups of 4). Core c handles batch c//4, heads
4*(c%4) .. 4*(c%4)+3 (feature slice of 256 columns). Wo is applied
row-parallel: each core emits a partial output; the host sums the 4
partials per batch (and untransposes). No collectives needed.

All matmuls run in float32r (TF32-like: ~1.5e-4 rel err on a K=1024
contraction, 4x the fp32 PE rate, full rate only when the moving free
dim is >=256). Raw fp32 bits are DMA'd directly into f32r tiles
(measured identical to an explicit rounding pass). On-chip f32r
operands (Q^T/K^T/V/P^T/out^T) are written by rounding copy/activation
producers as the walrus verifier requires.

Key structure decisions (all measured on HW):
 - everything is pre-transposed on the host so the kernel needs zero
   on-device transposes: xT (d,n), wqT/wkT/wvT (d,256), woT (256,d).
 - Q^T/K^T [feat, n] via dc-outer accumulation streaming behind the
   xT DMA; V in natural [n, feat] layout with a ones column appended
   (the ones column accumulates the softmax denominators during AV).
 - K^T is stored zero-padded per head to a full 128-row stationary:
   K=64 matmuls run at 2 cyc/row and read as low PE activity (HAM
   clock-gates to half speed); zero-padded K=128 runs at 1 cyc/row.
 - scores^T[k, q] (PE) -> exp via ScalarE reading 2 PSUM banks per
   call (the ACT engine is the pacing floor: n*n*heads/core exps at 1
   elem/cycle/lane) -> AV accumulates V_aug^T . P^T in PSUM [65, q].
 - passes are (q-half, head)-ordered and their emission is interleaved
   with the fc=1 projections so the PE fills ACT-paced slack; each
   q-half's output projection runs in the next half's ACT shadow.
 - normalize: copy avo out of PSUM fast (frees the accumulator), then
   reciprocal in a [128, 8] partition-scattered layout (a [1, 1024]
   row reciprocal is single-lane and 60x slower), partition_broadcast
   on GpSimd, multiply on DVE.
 - output projection keeps woT stationary (2 moving blocks per weight
   load) and emits the partial TRANSPOSED [d, n]; the host untransposes.

Biases are structurally zero in this problem spec and are skipped.
"""

import numpy as np

HEADS = 16
D = 1024
N = 2048
B = 2
N_CORES = 8
HPC = HEADS // (N_CORES // B)  # heads per core = 4
HD = D // HEADS                # 64
F = HPC * HD                   # 256 features per core
P = 128


def build_nc(n=N, d=D, hpc=HPC, hd=HD):
    """Build the per-core Bass program (SPMD: same program on all 8 cores)."""
    import concourse.bass as bass
    import concourse.tile as tile
    from concourse import bacc, mybir

    f32 = mybir.dt.float32
    f32r = mybir.dt.float32r
    f = hpc * hd            # per-core feature count (256)
    FC = f // P             # feature chunks (2)
    DC = d // P             # contraction chunks over d (8)
    NT = n // P             # n tiles / k chunks (16)
    QB = min(512, n)        # matmul moving block
    SCW = min(1024, n)      # scores psum width (2 banks)
    NSC = n // SCW          # q-halves
    scale = 1.0 / float(np.sqrt(np.float32(d)))

    nc = bacc.Bacc("TRN2")

    xT = nc.declare_dram_parameter("xT", [d, n], f32r, isOutput=False)
    wqT = nc.declare_dram_parameter("wqT", [d, f], f32r, isOutput=False)
    wkT = nc.declare_dram_parameter("wkT", [d, f], f32r, isOutput=False)
    wvT = nc.declare_dram_parameter("wvT", [d, f], f32r, isOutput=False)
    woT = nc.declare_dram_parameter("woT", [f, d], f32r, isOutput=False)
    out = nc.declare_dram_parameter("out", [d, n], f32, isOutput=True)

    xT_c = xT.rearrange("(c p) n -> c p n", p=P)
    wqT_c = wqT.rearrange("(c p) f -> c p f", p=P)
    wkT_c = wkT.rearrange("(c p) f -> c p f", p=P)
    wvT_c = wvT.rearrange("(c p) f -> c p f", p=P)
    woT_c = woT.rearrange("(c p) n -> c p n", p=P)

    with tile.TileContext(nc) as tc:
        with (
            tc.tile_pool(name="qkv", bufs=1) as qkv,
            tc.tile_pool(name="outT", bufs=1) as outp,
            # phase-2 pools created before the phase-1 pools so their
            # SBUF/PSUM ranges are disjoint: early heads' attention overlaps
            # the fc=1 projections with no pool-reuse serialization
            tc.tile_pool(name="pt", bufs=2) as ptp,
            tc.tile_pool(name="norm", bufs=1) as normp,
            tc.tile_pool(name="scps", bufs=2, space="PSUM") as scps,
            tc.tile_pool(name="avps", bufs=1, space="PSUM") as avps,
        ):
            QT_sb = qkv.tile([P, FC, n], f32r)
            # per-head K^T, zero-padded to a full 128-row stationary (head h
            # occupies partition rows po..po+hd, matching its rows in QT)
            KTz_sb = qkv.tile([P, hpc, n], f32r)
            V_sb = qkv.tile([P, NT, hpc, hd + 1], f32r)
            outT_sb = outp.tile([P, FC, n], f32r)
            # ones column of V_aug / zero fill of KTz: memset f32 consts, then
            # write via rounding DVE copies (direct memset on f32r fails
            # walrus codegen, and f32r matmul operands need rounding writers)
            ones_c = outp.tile([P, 1], f32)
            nc.vector.memset(ones_c[:], 1.0)
            nc.vector.tensor_copy(
                V_sb[:, :, :, hd : hd + 1],
                ones_c.to_broadcast([P, NT, hpc, 1]),
            )
            zero_c = outp.tile([P, 1], f32)
            nc.vector.memset(zero_c[:], 0.0)
            nc.vector.tensor_copy(
                KTz_sb[:], zero_c.to_broadcast([P, hpc, n])
            )

            def pass_begin():
                return avps.tile([hd + 1, SCW], f32, tag="avo", name="avo")

            def pass_blocks(avo, h, sh, kcs, pre_kc=None):
                """scores^T -> exp -> AV accumulate for k-chunks `kcs`."""
                fc = (h * hd) // P
                q0 = sh * SCW
                for kc in kcs:
                    if pre_kc is not None:
                        pre_kc(kc)
                    sc = scps.tile([P, SCW], f32, tag="sc")
                    for qc in range(SCW // QB):
                        nc.tensor.matmul(
                            sc[:, qc * QB : (qc + 1) * QB],
                            KTz_sb[:, h, kc * P : (kc + 1) * P],
                            QT_sb[:, fc, q0 + qc * QB : q0 + (qc + 1) * QB],
                            start=True,
                            stop=True,
                        )
                    pt = ptp.tile([P, SCW], f32r, tag="pt")
                    nc.scalar.activation(
                        pt[:], sc[:], mybir.ActivationFunctionType.Exp,
                        scale=scale,
                    )
                    for qc in range(SCW // QB):
                        nc.tensor.matmul(
                            avo[:, qc * QB : (qc + 1) * QB],
                            V_sb[:, kc, h, :],
                            pt[:, qc * QB : (qc + 1) * QB],
                            start=(kc == 0),
                            stop=(kc == NT - 1),
                        )

            def pass_end(avo, h, sh):
                """Free avo fast, then normalize rows 0..hd-1 by row hd (the
                softmax sums). reciprocal is single-lane-slow on a [1, SCW]
                row, so scatter the sums across partitions via a small SBUF
                DMA round-trip first."""
                fc = (h * hd) // P
                po = (h * hd) % P
                q0 = sh * SCW
                av_sb = normp.tile([hd + 1, SCW], f32, tag="av_sb")
                nc.vector.tensor_copy(av_sb[:], avo[:])
                rsh = normp.tile([P, SCW // P], f32, tag="rsh")
                nc.sync.dma_start(out=rsh[:], in_=av_sb[hd : hd + 1, :])
                rsh2 = normp.tile([P, SCW // P], f32, tag="rsh2")
                nc.vector.reciprocal(rsh2[:], rsh[:])
                recip = normp.tile([1, SCW], f32, tag="recip")
                nc.sync.dma_start(out=recip[:], in_=rsh2[:])
                bc = normp.tile([hd, SCW], f32, tag="bc")
                nc.gpsimd.partition_broadcast(bc[:], recip[:])
                nc.vector.tensor_mul(
                    outT_sb[po : po + hd, fc, q0 : q0 + SCW],
                    av_sb[0:hd, :],
                    bc[:],
                )

            def do_pass(h, sh, pre_kc=None):
                avo = pass_begin()
                pass_blocks(avo, h, sh, range(NT), pre_kc=pre_kc)
                pass_end(avo, h, sh)

            # ---- Phase 1 + first q-half heads 0/1, emission-interleaved ----
            with (
                tc.tile_pool(name="xw", bufs=1) as xw,
                tc.tile_pool(name="p1ps", bufs=2, space="PSUM") as p1ps,
            ):
                xT_r = xw.tile([P, DC, n], f32r)
                wqT_r = xw.tile([P, DC, f], f32r)
                wkT_r = xw.tile([P, DC, f], f32r)
                wvT_r = xw.tile([P, DC, f], f32r)

                # wq + xT interleaved per chunk: QT matmuls stream right
                # behind them; wk/wv stream during QT/KT compute.
                for dc in range(DC):
                    nc.sync.dma_start(out=wqT_r[:, dc, :], in_=wqT_c[dc])
                    nc.sync.dma_start(out=xT_r[:, dc, :], in_=xT_c[dc])

                def proj_cols(w_sb, is_k, fc, qcp):
                    # dc-outer accumulation, one sub-stage of 2 held banks
                    # covering moving columns [qcp*QB, (qcp+2)*QB)
                    pss = [
                        p1ps.tile([P, QB], f32, tag="big", name=f"pj{g}")
                        for g in range(2)
                    ]
                    for dc in range(DC):
                        for j in range(2):
                            qc = qcp + j
                            nc.tensor.matmul(
                                pss[j][:],
                                w_sb[:, dc, fc * P : (fc + 1) * P],
                                xT_r[:, dc, qc * QB : (qc + 1) * QB],
                                start=(dc == 0),
                                stop=(dc == DC - 1),
                            )
                    for j in range(2):
                        qc = qcp + j
                        sl = slice(qc * QB, (qc + 1) * QB)
                        if is_k:
                            # rows 0:64 = head 2fc (po=0), rows 64:128 =
                            # head 2fc+1 (po=64); keep row alignment
                            nc.vector.tensor_copy(
                                KTz_sb[0:hd, 2 * fc, sl], pss[j][0:hd, :]
                            )
                            nc.vector.tensor_copy(
                                KTz_sb[hd : 2 * hd, 2 * fc + 1, sl],
                                pss[j][hd : 2 * hd, :],
                            )
                        else:
                            nc.vector.tensor_copy(QT_sb[:, fc, sl], pss[j][:])

                def v_tile(nt):
                    ps = p1ps.tile([P, QB], f32, tag="big", name="vps")
                    for dc in range(DC):
                        nc.tensor.matmul(
                            ps[:, 0:f],
                            xT_r[:, dc, nt * P : (nt + 1) * P],
                            wvT_r[:, dc, :],
                            start=(dc == 0),
                            stop=(dc == DC - 1),
                        )
                    nc.vector.tensor_copy(
                        V_sb[:, nt, :, 0:hd],
                        ps[:, 0:f].rearrange("p (h e) -> p h e", h=hpc),
                    )

                # wk needed right after the first k0 sub-stage; wv by the
                # first v_tile — both AFTER the xT stream in queue order so
                # they don't delay the projection-gating xT chunks
                for dc in range(DC):
                    nc.sync.dma_start(out=wkT_r[:, dc, :], in_=wkT_c[dc])
                for dc in range(DC):
                    nc.sync.dma_start(out=wvT_r[:, dc, :], in_=wvT_c[dc])
                # Emission order = scheduling priority. Minimal chain to the
                # first exp: QT cols of the first q-half, then K^T in column
                # sub-stages interleaved with head 0's pass blocks (V tiles
                # interleaved per k-chunk they feed). Later projections are
                # emitted after the passes they should yield priority to, so
                # they fill the PE's ACT-paced slack.
                proj_cols(wqT_r, False, 0, 0)  # QT fc0 cols 0:1024 (q-half 0)
                avo0 = pass_begin()
                proj_cols(wkT_r, True, 0, 0)   # KTz fc0 cols 0:1024 (kc 0..7)
                pass_blocks(avo0, 0, 0, range(0, NT // 2), pre_kc=v_tile)
                proj_cols(wkT_r, True, 0, 2)   # KTz fc0 cols 1024:2048
                pass_blocks(avo0, 0, 0, range(NT // 2, NT), pre_kc=v_tile)
                pass_end(avo0, 0, 0)
                do_pass(1, 0)
                proj_cols(wqT_r, False, 0, 2)  # QT fc0 cols for q-half 1
                do_pass(0, 1)
                do_pass(1, 1)
                proj_cols(wqT_r, False, 1, 0)
                proj_cols(wqT_r, False, 1, 2)
                proj_cols(wkT_r, True, 1, 0)
                proj_cols(wkT_r, True, 1, 2)

            # ---- remaining passes + per-q-half output projection ----
            with (
                tc.tile_pool(name="wo", bufs=1) as wop,
                tc.tile_pool(name="wops", bufs=2, space="PSUM") as wopsp,
                tc.tile_pool(name="wosb", bufs=4) as wosbp,
            ):
                woT_sb = wop.tile([P, FC, d], f32r)
                for fc in range(FC):
                    nc.sync.dma_start(out=woT_sb[:, fc, :], in_=woT_c[fc])

                def wo_half(sh):
                    # output projection for q-half sh (woT stationary, 2
                    # moving q-blocks per weight load; emits partial^T [d, n])
                    q0 = sh * SCW
                    for do in range(d // P):
                        pss = [
                            wopsp.tile([P, QB], f32, tag="wops", name=f"wo{i}")
                            for i in range(SCW // QB)
                        ]
                        for fc in range(FC):
                            for qc in range(SCW // QB):
                                nc.tensor.matmul(
                                    pss[qc][:],
                                    woT_sb[:, fc, do * P : (do + 1) * P],
                                    outT_sb[
                                        :, fc, q0 + qc * QB : q0 + (qc + 1) * QB
                                    ],
                                    start=(fc == 0),
                                    stop=(fc == FC - 1),
                                )
                        for qc in range(SCW // QB):
                            ob = wosbp.tile([P, QB], f32, tag="ob")
                            nc.vector.tensor_copy(ob[:], pss[qc][:])
                            nc.sync.dma_start(
                                out=out[
                                    do * P : (do + 1) * P,
                                    q0 + qc * QB : q0 + (qc + 1) * QB,
                                ],
                                in_=ob[:],
                            )

                do_pass(2, 0)
                do_pass(3, 0)
                wo_half(0)
                do_pass(2, 1)
                do_pass(3, 1)
                wo_half(1)
    nc.finalize()
    return nc


def make_in_maps(x, Wq, Wk, Wv, Wo):
    """Shard full inputs into per-core DRAM parameter maps."""
    x = np.asarray(x, dtype=np.float32)
    Wq = np.asarray(Wq, dtype=np.float32)
    Wk = np.asarray(Wk, dtype=np.float32)
    Wv = np.asarray(Wv, dtype=np.float32)
    Wo = np.asarray(Wo, dtype=np.float32)
    xTs = [np.ascontiguousarray(x[b].T) for b in range(B)]
    WqT, WkT, WvT = Wq.T, Wk.T, Wv.T
    in_maps = []
    for c in range(N_CORES):
        b, g = c // (N_CORES // B), c % (N_CORES // B)
        fs = slice(g * F, (g + 1) * F)
        in_maps.append(
            {
                "xT": xTs[b],
                "wqT": np.ascontiguousarray(WqT[:, fs]),
                "wkT": np.ascontiguousarray(WkT[:, fs]),
                "wvT": np.ascontiguousarray(WvT[:, fs]),
                "woT": np.ascontiguousarray(Wo[:, fs].T),
            }
        )
    return in_maps


_NC_CACHE = {}


def _enable_ldw_opt():
    """Flip walrus --enable-ldw-opt to true: consecutive matmuls sharing a
    stationary operand skip the redundant LDWEIGHTS reload."""
    import concourse.bass_utils as bu

    if getattr(bu, "_ldw_opt_patched", False):
        return
    orig = bu.run_command

    def patched(argv, **kw):
        argv = [
            "--enable-ldw-opt=true" if a == "--enable-ldw-opt=false" else a
            for a in argv
        ]
        return orig(argv, **kw)

    bu.run_command = patched
    bu._ldw_opt_patched = True


def run(x, Wq, Wk, Wv, Wo, trace=False, **kw):
    from concourse.bass_utils import run_bass_kernel_spmd

    _enable_ldw_opt()
    if "nc" not in _NC_CACHE:
        _NC_CACHE["nc"] = build_nc()
    nc = _NC_CACHE["nc"]
    in_maps = make_in_maps(x, Wq, Wk, Wv, Wo)
    res = run_bass_kernel_spmd(
        nc, in_maps, core_ids=list(range(N_CORES)), trace=trace, **kw
    )
    parts = [np.asarray(res.results[i]["out"]) for i in range(N_CORES)]
    gpb = N_CORES // B
    # per-core partials are transposed [d, n]: sum the group, then untranspose
    full = np.stack(
        [
            sum(parts[b * gpb + 1 : (b + 1) * gpb], parts[b * gpb]).T
            for b in range(B)
        ]
    )
    return np.ascontiguousarray(full, dtype=np.float32), res


def kernel(x, Wq, bq, Wk, bk, Wv, bv, Wo, bo):
    full, _ = run(x, Wq, Wk, Wv, Wo)
    return full



# revision 8
# speedup vs baseline: 1.1528x; 1.1528x over previous
"""Multi-head attention kernel for 8 TRN2 NeuronCores.

Problem: b=2, n=2048, d=1024, heads=16, hd=64.
  q/k/v = x @ W{q,k,v}.T (+ zero bias)
  per head: softmax(q k^T / sqrt(d)) @ v
  out = concat @ Wo.T (+ zero bias)

Sharding (8 cores): data-parallel over batch (2) x tensor-parallel over
heads (16 heads -> 4 groups of 4). Core c handles batch c//4, heads
4*(c%4) .. 4*(c%4)+3 (feature slice of 256 columns). Wo is applied
row-parallel: each core emits a partial output; the host sums the 4
partials per batch (and untransposes). No collectives needed.

v2 design (vs the f32r baseline at ~267us):
 - Everything bf16 (same PE rate as f32r, half the DMA + SBUF, and
   bf16 PSUM scores take half the banks). Host pre-transposes and
   pre-casts: xT (d,n), wqT/wkT/wvT (d,256), woT (256,d), all bf16.
 - Scores are ROW-TILED: the head pair of a 128-feature chunk lives in
   partition halves 0-63 / 64-127 (the natural projection layout), and
   the two K=64 matmuls run concurrently in different PE row-groups
   (tile_position auto-derived from base partitions). No zero-padding,
   half the PE time of the padded-K=128 baseline.
 - exp batching: one ACTIVATE covers 2 kc-chunks x 2 heads x 512 q =
   2048 elems/lane, 64 calls total (vs 128) -> ~19us less ACT overhead.
 - PSUM budget: scps 2 bufs x 2 banks (bf16) + avo pair 2 banks +
   2 banks for projections/Wo. Projections are emitted between passes
   and fill the PE slack of the ACT-paced attention steady state.
 - Loop: q-block outer (4 x 512), head-pair inner; wo(qb) emitted after
   each q-block so it fills the next block's ACT shadow; softmax
   denominators via the ones-column of V_aug (row 64 of the AV psum).
 - normalize: copy avo out of PSUM fast, reciprocal in a [128, 4]
   partition-scattered layout (single-lane [1,512] reciprocal is slow),
   partition_broadcast on GpSimd, multiply on DVE -> outT bf16.

Biases are structurally zero in this problem spec and are skipped.
"""

import numpy as np

HEADS = 16
D = 1024
N = 2048
B = 2
N_CORES = 8
HPC = HEADS // (N_CORES // B)  # heads per core = 4
HD = D // HEADS                # 64
F = HPC * HD                   # 256 features per core
P = 128


def build_nc(n=N, d=D, hpc=HPC, hd=HD):
    """Build the per-core Bass program (SPMD: same program on all 8 cores)."""
    import concourse.bass as bass
    import concourse.tile as tile
    from concourse import bacc, mybir

    f32 = mybir.dt.float32
    bf16 = mybir.dt.bfloat16
    f = hpc * hd            # per-core feature count (256)
    FC = f // P             # feature chunks / head pairs (2)
    DC = d // P             # contraction chunks over d (8)
    NT = n // P             # key chunks (16)
    QB = 512                # q-block width
    NQB = n // QB           # 4
    scale = 1.0 / float(np.sqrt(np.float32(d)))

    nc = bacc.Bacc("TRN2")

    xT = nc.declare_dram_parameter("xT", [d, n], bf16, isOutput=False)
    wqT = nc.declare_dram_parameter("wqT", [d, f], bf16, isOutput=False)
    wkT = nc.declare_dram_parameter("wkT", [d, f], bf16, isOutput=False)
    wvT = nc.declare_dram_parameter("wvT", [d, f], bf16, isOutput=False)
    woT = nc.declare_dram_parameter("woT", [f, d], bf16, isOutput=False)
    out = nc.declare_dram_parameter("out", [d, n], bf16, isOutput=True)

    xT_c = xT.rearrange("(c p) n -> c p n", p=P)
    wqT_c = wqT.rearrange("(c p) f -> c p f", p=P)
    wkT_c = wkT.rearrange("(c p) f -> c p f", p=P)
    wvT_c = wvT.rearrange("(c p) f -> c p f", p=P)
    woT_c = woT.rearrange("(c p) n -> c p n", p=P)

    with tile.TileContext(nc) as tc:
        with (
            tc.tile_pool(name="qkv", bufs=1) as qkv,
            tc.tile_pool(name="outT", bufs=1) as outp,
            tc.tile_pool(name="pt", bufs=2) as ptp,
            tc.tile_pool(name="norm", bufs=1) as normp,
            tc.tile_pool(name="xw", bufs=1) as xw,
            tc.tile_pool(name="wosb", bufs=4) as wosbp,
            tc.tile_pool(name="scps", bufs=2, space="PSUM") as scps,
            tc.tile_pool(name="avps", bufs=1, space="PSUM") as avps,
            tc.tile_pool(name="pjps", bufs=2, space="PSUM") as pjps,
        ):
            QT_sb = qkv.tile([P, FC, n], bf16)
            KT_sb = qkv.tile([P, FC, n], bf16)
            V_sb = qkv.tile([P, NT, hpc, hd + 1], bf16)
            outT_sb = outp.tile([P, FC, n], bf16)
            woT_sb = outp.tile([P, FC, d], bf16)
            # ones column of V_aug (accumulates softmax denominators in AV)
            ones_c = outp.tile([P, 1], bf16)
            nc.vector.memset(ones_c[:], 1.0)
            nc.vector.tensor_copy(
                V_sb[:, :, :, hd : hd + 1],
                ones_c.to_broadcast([P, NT, hpc, 1]),
            )

            xT_r = xw.tile([P, DC, n], bf16)
            wqT_r = xw.tile([P, DC, f], bf16)
            wkT_r = xw.tile([P, DC, f], bf16)
            wvT_r = xw.tile([P, DC, f], bf16)

            # wk + xT interleaved per chunk: KT matmuls stream right
            # behind them; wq/wv stream during KT/QT compute.
            for dc in range(DC):
                nc.sync.dma_start(out=wkT_r[:, dc, :], in_=wkT_c[dc])
                nc.sync.dma_start(out=xT_r[:, dc, :], in_=xT_c[dc])
            for dc in range(DC):
                nc.sync.dma_start(out=wqT_r[:, dc, :], in_=wqT_c[dc])
            for dc in range(DC):
                nc.sync.dma_start(out=wvT_r[:, dc, :], in_=wvT_c[dc])
            for fc in range(FC):
                nc.sync.dma_start(out=woT_sb[:, fc, :], in_=woT_c[fc])

            def proj_qk(w_sb, dest, fc, qc):
                """One 512-col block of the Q or K projection (dc-outer
                accumulation), written to dest[:, fc, qc*QB:...] as bf16."""
                ps = pjps.tile([P, QB], f32, tag="pj")
                sl = slice(qc * QB, (qc + 1) * QB)
                for dc in range(DC):
                    nc.tensor.matmul(
                        ps[:],
                        w_sb[:, dc, fc * P : (fc + 1) * P],
                        xT_r[:, dc, sl],
                        start=(dc == 0),
                        stop=(dc == DC - 1),
                    )
                nc.vector.tensor_copy(dest[:, fc, sl], ps[:])

            def v_tile(nt, fc):
                """V for key-chunk nt, heads 2fc/2fc+1 (natural [keys, feat]
                layout; ones column at hd written at init)."""
                ps = pjps.tile([P, QB], f32, tag="pj")
                nc.tensor.matmul(
                    ps[:, 0:P],
                    xT_r[:, 0, nt * P : (nt + 1) * P],
                    wvT_r[:, 0, fc * P : (fc + 1) * P],
                    start=True,
                    stop=False,
                )
                for dc in range(1, DC):
                    nc.tensor.matmul(
                        ps[:, 0:P],
                        xT_r[:, dc, nt * P : (nt + 1) * P],
                        wvT_r[:, dc, fc * P : (fc + 1) * P],
                        start=False,
                        stop=(dc == DC - 1),
                    )
                nc.vector.tensor_copy(
                    V_sb[:, nt, 2 * fc : 2 * fc + 2, 0:hd],
                    ps[:, 0:P].rearrange("p (h e) -> p h e", h=2),
                )

            def do_pass(fc, qb, pre_round=None):
                """Attention pass for head pair fc (heads 2fc, 2fc+1) on
                q-block qb: 16 rounds of (2 heads row-tiled scores ->
                one FD=1024 exp -> 2 AV accumulation matmuls)."""
                q0 = qb * QB
                avos = [
                    avps.tile([hd + 1, QB], f32, tag=f"avo{i}", name=f"avo{i}")
                    for i in range(2)
                ]
                for kc in range(NT):
                    if pre_round is not None:
                        pre_round(kc)
                    sc = scps.tile([P, 2, QB], f32, tag="sc")
                    for hi in range(2):
                        p0 = hi * hd
                        nc.tensor.matmul(
                            sc[:, hi, :],
                            KT_sb[p0 : p0 + hd, fc, kc * P : (kc + 1) * P],
                            QT_sb[p0 : p0 + hd, fc, q0 : q0 + QB],
                            start=True,
                            stop=True,
                        )
                    pt = ptp.tile([P, 2, QB], bf16, tag="pt")
                    nc.scalar.activation(
                        pt[:], sc[:], mybir.ActivationFunctionType.Exp,
                        scale=scale,
                    )
                    for hi in range(2):
                        nc.tensor.matmul(
                            avos[hi][:],
                            V_sb[:, kc, 2 * fc + hi, :],
                            pt[:, hi, :],
                            start=(kc == 0),
                            stop=(kc == NT - 1),
                        )
                pass_end(fc, qb, avos)

            def pass_end(fc, qb, avos):
                """Free avo fast, then normalize rows 0..hd-1 by row hd (the
                softmax sums). reciprocal is single-lane-slow on a [1, QB]
                row, so scatter the sums across partitions via a small SBUF
                DMA round-trip first."""
                q0 = qb * QB
                for hi in range(2):
                    po = hi * hd
                    av_sb = normp.tile([hd + 1, QB], f32, tag=f"av_sb{hi}")
                    nc.vector.tensor_copy(av_sb[:], avos[hi][:])
                    rsh = normp.tile([P, QB // P], f32, tag=f"rsh{hi}")
                    nc.sync.dma_start(out=rsh[:], in_=av_sb[hd : hd + 1, :])
                    rsh2 = normp.tile([P, QB // P], f32, tag=f"rsh2{hi}")
                    nc.vector.reciprocal(rsh2[:], rsh[:])
                    recip = normp.tile([1, QB], f32, tag=f"recip{hi}")
                    nc.sync.dma_start(out=recip[:], in_=rsh2[:])
                    bc = normp.tile([hd, QB], f32, tag=f"bc{hi}")
                    nc.gpsimd.partition_broadcast(bc[:], recip[:])
                    nc.vector.tensor_mul(
                        outT_sb[po : po + hd, fc, q0 : q0 + QB],
                        av_sb[0:hd, :],
                        bc[:],
                    )

            def wo_block(qb):
                """Output projection for q-block qb (woT stationary, emits
                the partial TRANSPOSED [d, QB]; the host untransposes)."""
                q0 = qb * QB
                for do in range(d // P):
                    ps = pjps.tile([P, QB], f32, tag="pj", name=f"wo{do % 2}")
                    for fc in range(FC):
                        nc.tensor.matmul(
                            ps[:],
                            woT_sb[:, fc, do * P : (do + 1) * P],
                            outT_sb[:, fc, q0 : q0 + QB],
                            start=(fc == 0),
                            stop=(fc == FC - 1),
                        )
                    ob = wosbp.tile([P, QB], bf16, tag="ob")
                    nc.vector.tensor_copy(ob[:], ps[:])
                    nc.sync.dma_start(
                        out=out[do * P : (do + 1) * P, q0 : q0 + QB],
                        in_=ob[:],
                    )

            # ---- emission order = scheduling priority ----
            # Minimal chain to the first exp: KT fc0 (full n, gates every
            # round of pass(0,0)) + QT fc0 block 0. V tiles for fc0 stream
            # per-round inside pass(0,0); fc1's K/Q/V are emitted after
            # pass(0,0) so they fill its ACT shadow, and later projections
            # fill later passes' shadows.
            for qc in range(NQB):
                proj_qk(wkT_r, KT_sb, 0, qc)
            proj_qk(wqT_r, QT_sb, 0, 0)

            do_pass(0, 0, pre_round=lambda kc: v_tile(kc, 0))

            for qc in range(NQB):
                proj_qk(wkT_r, KT_sb, 1, qc)
            proj_qk(wqT_r, QT_sb, 1, 0)

            do_pass(1, 0, pre_round=lambda kc: v_tile(kc, 1))

            for qb in range(1, NQB):
                wo_block(qb - 1)
                proj_qk(wqT_r, QT_sb, 0, qb)
                do_pass(0, qb)
                proj_qk(wqT_r, QT_sb, 1, qb)
                do_pass(1, qb)
            wo_block(NQB - 1)
    nc.finalize()
    return nc


def make_in_maps(x, Wq, Wk, Wv, Wo):
    """Shard full inputs into per-core DRAM parameter maps (bf16)."""
    import ml_dtypes

    bf = ml_dtypes.bfloat16
    x = np.asarray(x, dtype=np.float32)
    Wq = np.asarray(Wq, dtype=np.float32)
    Wk = np.asarray(Wk, dtype=np.float32)
    Wv = np.asarray(Wv, dtype=np.float32)
    Wo = np.asarray(Wo, dtype=np.float32)
    xTs = [np.ascontiguousarray(x[b].T).astype(bf) for b in range(B)]
    WqT, WkT, WvT = Wq.T, Wk.T, Wv.T
    in_maps = []
    for c in range(N_CORES):
        b, g = c // (N_CORES // B), c % (N_CORES // B)
        fs = slice(g * F, (g + 1) * F)
        in_maps.append(
            {
                "xT": xTs[b],
                "wqT": np.ascontiguousarray(WqT[:, fs]).astype(bf),
                "wkT": np.ascontiguousarray(WkT[:, fs]).astype(bf),
                "wvT": np.ascontiguousarray(WvT[:, fs]).astype(bf),
                "woT": np.ascontiguousarray(Wo[:, fs].T).astype(bf),
            }
        )
    return in_maps


_NC_CACHE = {}


def _enable_ldw_opt():
    """Flip walrus --enable-ldw-opt to true: consecutive matmuls sharing a
    stationary operand skip the redundant LDWEIGHTS reload."""
    import concourse.bass_utils as bu

    if getattr(bu, "_ldw_opt_patched", False):
        return
    orig = bu.run_command

    def patched(argv, **kw):
        argv = [
            "--enable-ldw-opt=true" if a == "--enable-ldw-opt=false" else a
            for a in argv
        ]
        return orig(argv, **kw)

    bu.run_command = patched
    bu._ldw_opt_patched = True


def run(x, Wq, Wk, Wv, Wo, trace=False, **kw):
    from concourse.bass_utils import run_bass_kernel_spmd

    if "nc" not in _NC_CACHE:
        _NC_CACHE["nc"] = build_nc()
    nc = _NC_CACHE["nc"]
    in_maps = make_in_maps(x, Wq, Wk, Wv, Wo)
    res = run_bass_kernel_spmd(
        nc, in_maps, core_ids=list(range(N_CORES)), trace=trace, **kw
    )
    parts = [
        np.asarray(res.results[i]["out"]).astype(np.float32)
        for i in range(N_CORES)
    ]
    gpb = N_CORES // B
    # per-core partials are transposed [d, n]: sum the group, then untranspose
    full = np.stack(
        [
            sum(parts[b * gpb + 1 : (b + 1) * gpb], parts[b * gpb]).T
            for b in range(B)
        ]
    )
    return np.ascontiguousarray(full, dtype=np.float32), res


def kernel(x, Wq, bq, Wk, bk, Wv, bv, Wo, bo):
    full, _ = run(x, Wq, Wk, Wv, Wo)
    return full


# revision 9
# speedup vs baseline: 1.3271x; 1.1512x over previous
"""Multi-head attention kernel for 8 TRN2 NeuronCores.

Problem: b=2, n=2048, d=1024, heads=16, hd=64.
  q/k/v = x @ W{q,k,v}.T (+ zero bias)
  per head: softmax(q k^T / sqrt(d)) @ v
  out = concat @ Wo.T (+ zero bias)

Sharding (8 cores): data-parallel over batch (2) x tensor-parallel over
heads (16 heads -> 4 groups of 4). Core c handles batch c//4, heads
4*(c%4) .. 4*(c%4)+3 (feature slice of 256 columns). Wo is applied
row-parallel: each core emits a partial output (transposed [d, n],
bf16); the host sums the 4 partials per batch and untransposes.

v3 design (measured HW evolution from the f32r baseline at ~267us and
the v2 rewrite at ~270us):
 - Everything bf16 (same PE rate as f32r, half the DMA + SBUF). Host
   pre-transposes/casts: xT (d,n), w{q,k,v}T (d,256), woT (256,d).
 - Steady state is ACT-paced: per round (head-pair, 512-q block, one
   128-key chunk) the PE does 2 scores matmuls (row-tiled K=64 pair,
   ~390ns) + 2 AV matmuls (~430ns) against one FD=1024 exp (~1114ns).
   The ~365ns/round PE slack is filled by "work items" (projection
   half-blocks, V tiles, Wo chunks) emitted BETWEEN the scores and AV
   matmuls of each round - engine queues are FIFO, so emission order
   controls exactly what the PE does during the exp shadow.
 - Minimal lead-in: only K^T fc0 block0 (streamed behind the xT DMA)
   + Q^T fc0 qb0 + V(0..1) gate the first exp; all other projections
   ride inside passes as work items.
 - Wo for q-block qb is emitted as per-do items inside the next
   q-block's first pass (round >= 4, giving the normalize chain time);
   the last q-block's Wo runs at the tail with its PSUM->SBUF casts on
   ScalarE (idle there) instead of DVE.
 - softmax denominators via the ones-column of V_aug (row hd of avo);
   normalize: copy avo out of PSUM fast, reciprocal in a [128, 4]
   partition-scattered layout via a small SBUF DMA round-trip,
   partition_broadcast on GpSimd, multiply on DVE -> outT bf16.
 - PSUM: scps 2 bufs x [128,2,512]f32 (2 banks each) + avo pair
   (2 banks) + pjps 2 bufs x 1 bank for projection/Wo items = 8 banks.

Biases are structurally zero in this problem spec and are skipped.
"""

import numpy as np

HEADS = 16
D = 1024
N = 2048
B = 2
N_CORES = 8
HPC = HEADS // (N_CORES // B)  # heads per core = 4
HD = D // HEADS                # 64
F = HPC * HD                   # 256 features per core
P = 128


def build_nc(n=N, d=D, hpc=HPC, hd=HD):
    """Build the per-core Bass program (SPMD: same program on all 8 cores)."""
    import concourse.bass as bass
    import concourse.tile as tile
    from concourse import bacc, mybir

    f32 = mybir.dt.float32
    bf16 = mybir.dt.bfloat16
    f = hpc * hd            # per-core feature count (256)
    FC = f // P             # feature chunks / head pairs (2)
    DC = d // P             # contraction chunks over d (8)
    NT = n // P             # key chunks (16)
    QB = 512                # q-block width
    NQB = n // QB           # 4
    scale = 1.0 / float(np.sqrt(np.float32(d)))

    nc = bacc.Bacc("TRN2")

    xT = nc.declare_dram_parameter("xT", [d, n], bf16, isOutput=False)
    wqT = nc.declare_dram_parameter("wqT", [d, f], bf16, isOutput=False)
    wkT = nc.declare_dram_parameter("wkT", [d, f], bf16, isOutput=False)
    wvT = nc.declare_dram_parameter("wvT", [d, f], bf16, isOutput=False)
    woT = nc.declare_dram_parameter("woT", [f, d], bf16, isOutput=False)
    out = nc.declare_dram_parameter("out", [d, n], bf16, isOutput=True)

    xT_c = xT.rearrange("(c p) n -> c p n", p=P)
    wqT_c = wqT.rearrange("(c p) f -> c p f", p=P)
    wkT_c = wkT.rearrange("(c p) f -> c p f", p=P)
    wvT_c = wvT.rearrange("(c p) f -> c p f", p=P)
    woT_c = woT.rearrange("(c p) n -> c p n", p=P)

    with tile.TileContext(nc) as tc:
        with (
            tc.tile_pool(name="qkv", bufs=1) as qkv,
            tc.tile_pool(name="outT", bufs=1) as outp,
            tc.tile_pool(name="pt", bufs=2) as ptp,
            tc.tile_pool(name="norm", bufs=1) as normp,
            tc.tile_pool(name="xw", bufs=1) as xw,
            tc.tile_pool(name="wosb", bufs=4) as wosbp,
            tc.tile_pool(name="scps", bufs=2, space="PSUM") as scps,
            tc.tile_pool(name="avps", bufs=1, space="PSUM") as avps,
            tc.tile_pool(name="pjps", bufs=2, space="PSUM") as pjps,
        ):
            QT_sb = qkv.tile([P, FC, n], bf16)
            KT_sb = qkv.tile([P, FC, n], bf16)
            V_sb = qkv.tile([P, NT, hpc, hd + 1], bf16)
            outT_sb = outp.tile([P, FC, n], bf16)
            woT_sb = outp.tile([P, FC, d], bf16)
            # ones column of V_aug (accumulates softmax denominators in AV)
            ones_c = outp.tile([P, 1], bf16)
            nc.vector.memset(ones_c[:], 1.0)
            nc.vector.tensor_copy(
                V_sb[:, :, :, hd : hd + 1],
                ones_c.to_broadcast([P, NT, hpc, 1]),
            )

            xT_r = xw.tile([P, DC, n], bf16)
            wqT_r = xw.tile([P, DC, f], bf16)
            wkT_r = xw.tile([P, DC, f], bf16)
            wvT_r = xw.tile([P, DC, f], bf16)

            # wk + xT interleaved per chunk: the KT block-0 matmuls stream
            # right behind them; wq/wv arrive during KT/QT compute.
            for dc in range(DC):
                nc.sync.dma_start(out=wkT_r[:, dc, :], in_=wkT_c[dc])
                nc.sync.dma_start(out=xT_r[:, dc, :], in_=xT_c[dc])
            for dc in range(DC):
                nc.sync.dma_start(out=wqT_r[:, dc, :], in_=wqT_c[dc])
            for dc in range(DC):
                nc.sync.dma_start(out=wvT_r[:, dc, :], in_=wvT_c[dc])
            for fc in range(FC):
                nc.sync.dma_start(out=woT_sb[:, fc, :], in_=woT_c[fc])

            # ---- work items: each emits <=~900ns of PE work; projection
            # blocks are split into two 4-dc halves sharing one pjps tile ----
            half_state = {}

            def qk_half(w_sb, dest, fc, qc, half):
                key = (id(w_sb), fc, qc)
                sl = slice(qc * QB, (qc + 1) * QB)
                if half == 0:
                    ps = pjps.tile([P, QB], f32, tag="pj")
                    half_state[key] = ps
                else:
                    ps = half_state.pop(key)
                for dc in range(half * 4, half * 4 + 4):
                    nc.tensor.matmul(
                        ps[:],
                        w_sb[:, dc, fc * P : (fc + 1) * P],
                        xT_r[:, dc, sl],
                        start=(dc == 0),
                        stop=(dc == DC - 1),
                    )
                if half == 1:
                    nc.vector.tensor_copy(dest[:, fc, sl], ps[:])

            def v_half(nt, half):
                """V for key-chunk nt, all 4 heads (N=256 moving)."""
                key = ("v", nt)
                if half == 0:
                    ps = pjps.tile([P, QB], f32, tag="pj")
                    half_state[key] = ps
                else:
                    ps = half_state.pop(key)
                for dc in range(half * 4, half * 4 + 4):
                    nc.tensor.matmul(
                        ps[:, 0:f],
                        xT_r[:, dc, nt * P : (nt + 1) * P],
                        wvT_r[:, dc, :],
                        start=(dc == 0),
                        stop=(dc == DC - 1),
                    )
                if half == 1:
                    nc.vector.tensor_copy(
                        V_sb[:, nt, :, 0:hd],
                        ps[:, 0:f].rearrange("p (h e) -> p h e", h=hpc),
                    )

            def wo_item(qb, do, tail=False):
                """One do-chunk of the output projection for q-block qb
                (woT stationary; emits the partial TRANSPOSED [128, QB])."""
                q0 = qb * QB
                ps = pjps.tile([P, QB], f32, tag="pj")
                for fc in range(FC):
                    nc.tensor.matmul(
                        ps[:],
                        woT_sb[:, fc, do * P : (do + 1) * P],
                        outT_sb[:, fc, q0 : q0 + QB],
                        start=(fc == 0),
                        stop=(fc == FC - 1),
                    )
                ob = wosbp.tile([P, QB], bf16, tag="ob")
                if tail:
                    nc.scalar.copy(ob[:], ps[:])
                else:
                    nc.vector.tensor_copy(ob[:], ps[:])
                nc.sync.dma_start(
                    out=out[do * P : (do + 1) * P, q0 : q0 + QB],
                    in_=ob[:],
                )

            def pass_end(fc, qb, avos):
                """Free avo fast, then normalize rows 0..hd-1 by row hd (the
                softmax sums). reciprocal is single-lane-slow on a [1, QB]
                row, so scatter the sums across partitions via a small SBUF
                DMA round-trip first."""
                q0 = qb * QB
                for hi in range(2):
                    po = hi * hd
                    av_sb = normp.tile([hd + 1, QB], f32, tag=f"av_sb{hi}")
                    nc.vector.tensor_copy(av_sb[:], avos[hi][:])
                    rsh = normp.tile([P, QB // P], f32, tag=f"rsh{hi}")
                    nc.sync.dma_start(out=rsh[:], in_=av_sb[hd : hd + 1, :])
                    rsh2 = normp.tile([P, QB // P], f32, tag=f"rsh2{hi}")
                    nc.vector.reciprocal(rsh2[:], rsh[:])
                    recip = normp.tile([1, QB], f32, tag=f"recip{hi}")
                    nc.sync.dma_start(out=recip[:], in_=rsh2[:])
                    bc = normp.tile([hd, QB], f32, tag=f"bc{hi}")
                    nc.gpsimd.partition_broadcast(bc[:], recip[:])
                    nc.vector.tensor_mul(
                        outT_sb[po : po + hd, fc, q0 : q0 + QB],
                        av_sb[0:hd, :],
                        bc[:],
                    )

            def do_pass(fc, qb, fillers):
                """Attention pass for head pair fc (heads 2fc, 2fc+1) on
                q-block qb. fillers[r] = list of work items emitted between
                round r's scores and AV matmuls (they run in the exp
                shadow; the engine queue is FIFO so placement = pacing)."""
                q0 = qb * QB
                avos = [
                    avps.tile([hd + 1, QB], f32, tag=f"avo{i}", name=f"avo{i}")
                    for i in range(2)
                ]
                for kc in range(NT):
                    sc = scps.tile([P, 2, QB], f32, tag="sc")
                    for hi in range(2):
                        p0 = hi * hd
                        nc.tensor.matmul(
                            sc[:, hi, :],
                            KT_sb[p0 : p0 + hd, fc, kc * P : (kc + 1) * P],
                            QT_sb[p0 : p0 + hd, fc, q0 : q0 + QB],
                            start=True,
                            stop=True,
                        )
                    pt = ptp.tile([P, 2, QB], bf16, tag="pt")
                    nc.scalar.activation(
                        pt[:], sc[:], mybir.ActivationFunctionType.Exp,
                        scale=scale,
                    )
                    for item in fillers.get(kc, []):
                        item()
                    for hi in range(2):
                        nc.tensor.matmul(
                            avos[hi][:],
                            V_sb[:, kc, 2 * fc + hi, :],
                            pt[:, hi, :],
                            start=(kc == 0),
                            stop=(kc == NT - 1),
                        )
                pass_end(fc, qb, avos)

            # ---- emission schedule ----
            def QK(w, dest, fc, qc, half):
                return lambda: qk_half(w, dest, fc, qc, half)

            def VH(nt, half):
                return lambda: v_half(nt, half)

            def WO(qb, do, tail=False):
                return lambda: wo_item(qb, do, tail)

            # lead-in: minimal chain to the first exp
            qk_half(wkT_r, KT_sb, 0, 0, 0)
            qk_half(wkT_r, KT_sb, 0, 0, 1)
            qk_half(wqT_r, QT_sb, 0, 0, 0)
            qk_half(wqT_r, QT_sb, 0, 0, 1)
            v_half(0, 0)
            v_half(0, 1)
            v_half(1, 0)
            v_half(1, 1)

            # pass(0,0): carries V(2..15), KT fc0 b1-b3 (b_j needed by round
            # 4j), KT fc1 b0 and QT fc1 qb0 (needed by pass(1,0) round 0).
            f00 = {}
            for kc in range(14):
                f00[kc] = [VH(kc + 2, 0), VH(kc + 2, 1)]
            for i, it in enumerate(
                [QK(wkT_r, KT_sb, 0, 1, 0), QK(wkT_r, KT_sb, 0, 1, 1),
                 QK(wkT_r, KT_sb, 0, 2, 0), QK(wkT_r, KT_sb, 0, 2, 1),
                 QK(wkT_r, KT_sb, 0, 3, 0), QK(wkT_r, KT_sb, 0, 3, 1),
                 QK(wkT_r, KT_sb, 1, 0, 0), QK(wkT_r, KT_sb, 1, 0, 1),
                 QK(wqT_r, QT_sb, 1, 0, 0), QK(wqT_r, QT_sb, 1, 0, 1)]
            ):
                f00.setdefault(i + 2, []).append(it)
            do_pass(0, 0, f00)

            # pass(1,0): carries KT fc1 b1-b3 (b_j by round 4j) + QT qb1.
            f10 = {}
            for i, it in enumerate(
                [QK(wkT_r, KT_sb, 1, 1, 0), QK(wkT_r, KT_sb, 1, 1, 1),
                 QK(wkT_r, KT_sb, 1, 2, 0), QK(wkT_r, KT_sb, 1, 2, 1),
                 QK(wkT_r, KT_sb, 1, 3, 0), QK(wkT_r, KT_sb, 1, 3, 1),
                 QK(wqT_r, QT_sb, 0, 1, 0), QK(wqT_r, QT_sb, 0, 1, 1),
                 QK(wqT_r, QT_sb, 1, 1, 0), QK(wqT_r, QT_sb, 1, 1, 1)]
            ):
                f10.setdefault(i, []).append(it)
            do_pass(1, 0, f10)

            # q-blocks 1..3: wo(qb-1) rides in pass(0, qb) from round 4
            # (after the qb-1 normalize chains land); the next q-block's
            # QT blocks ride in pass(1, qb).
            for qb in range(1, NQB):
                fa = {}
                for do in range(d // P):
                    fa.setdefault(4 + do, []).append(WO(qb - 1, do))
                do_pass(0, qb, fa)
                fb = {}
                if qb < NQB - 1:
                    for i, it in enumerate(
                        [QK(wqT_r, QT_sb, 0, qb + 1, 0),
                         QK(wqT_r, QT_sb, 0, qb + 1, 1),
                         QK(wqT_r, QT_sb, 1, qb + 1, 0),
                         QK(wqT_r, QT_sb, 1, qb + 1, 1)]
                    ):
                        fb.setdefault(2 * i, []).append(it)
                do_pass(1, qb, fb)
            for do in range(d // P):
                wo_item(NQB - 1, do, tail=True)
    nc.finalize()
    return nc


def make_in_maps(x, Wq, Wk, Wv, Wo):
    """Shard full inputs into per-core DRAM parameter maps (bf16)."""
    import ml_dtypes

    bf = ml_dtypes.bfloat16
    x = np.asarray(x, dtype=np.float32)
    Wq = np.asarray(Wq, dtype=np.float32)
    Wk = np.asarray(Wk, dtype=np.float32)
    Wv = np.asarray(Wv, dtype=np.float32)
    Wo = np.asarray(Wo, dtype=np.float32)
    xTs = [np.ascontiguousarray(x[b].T).astype(bf) for b in range(B)]
    WqT, WkT, WvT = Wq.T, Wk.T, Wv.T
    in_maps = []
    for c in range(N_CORES):
        b, g = c // (N_CORES // B), c % (N_CORES // B)
        fs = slice(g * F, (g + 1) * F)
        in_maps.append(
            {
                "xT": xTs[b],
                "wqT": np.ascontiguousarray(WqT[:, fs]).astype(bf),
                "wkT": np.ascontiguousarray(WkT[:, fs]).astype(bf),
                "wvT": np.ascontiguousarray(WvT[:, fs]).astype(bf),
                "woT": np.ascontiguousarray(Wo[:, fs].T).astype(bf),
            }
        )
    return in_maps


_NC_CACHE = {}


def run(x, Wq, Wk, Wv, Wo, trace=False, **kw):
    from concourse.bass_utils import run_bass_kernel_spmd

    if "nc" not in _NC_CACHE:
        _NC_CACHE["nc"] = build_nc()
    nc = _NC_CACHE["nc"]
    in_maps = make_in_maps(x, Wq, Wk, Wv, Wo)
    res = run_bass_kernel_spmd(
        nc, in_maps, core_ids=list(range(N_CORES)), trace=trace, **kw
    )
    parts = [
        np.asarray(res.results[i]["out"]).astype(np.float32)
        for i in range(N_CORES)
    ]
    gpb = N_CORES // B
    # per-core partials are transposed [d, n]: sum the group, then untranspose
    full = np.stack(
        [
            sum(parts[b * gpb + 1 : (b + 1) * gpb], parts[b * gpb]).T
            for b in range(B)
        ]
    )
    return np.ascontiguousarray(full, dtype=np.float32), res


def kernel(x, Wq, bq, Wk, bk, Wv, bv, Wo, bo):
    full, _ = run(x, Wq, Wk, Wv, Wo)
    return full


# revision 12
# speedup vs baseline: 1.3517x; 1.0186x over previous
"""Multi-head attention kernel for 8 TRN2 NeuronCores.

Problem: b=2, n=2048, d=1024, heads=16, hd=64.
  q/k/v = x @ W{q,k,v}.T (+ zero bias)
  per head: softmax(q k^T / sqrt(d)) @ v
  out = concat @ Wo.T (+ zero bias)

Sharding (8 cores): data-parallel over batch (2) x tensor-parallel over
heads (16 heads -> 4 groups of 4). Core c handles batch c//4, heads
4*(c%4) .. 4*(c%4)+3 (feature slice of 256 columns). Wo is applied
row-parallel: each core emits a partial output (transposed [d, n],
bf16); the host sums the 4 partials per batch and untransposes.

v3 design (measured HW evolution from the f32r baseline at ~267us and
the v2 rewrite at ~270us):
 - Everything bf16 (same PE rate as f32r, half the DMA + SBUF). Host
   pre-transposes/casts: xT (d,n), w{q,k,v}T (d,256), woT (256,d).
 - Steady state is ACT-paced: per round (head-pair, 512-q block, one
   128-key chunk) the PE does 2 scores matmuls (row-tiled K=64 pair,
   ~390ns) + 2 AV matmuls (~430ns) against one FD=1024 exp (~1114ns).
   The ~365ns/round PE slack is filled by "work items" (projection
   half-blocks, V tiles, Wo chunks) emitted BETWEEN the scores and AV
   matmuls of each round - engine queues are FIFO, so emission order
   controls exactly what the PE does during the exp shadow.
 - Minimal lead-in: only K^T fc0 block0 (streamed behind the xT DMA)
   + Q^T fc0 qb0 + V(0..1) gate the first exp; all other projections
   ride inside passes as work items.
 - Wo for q-block qb is emitted as per-do items inside the next
   q-block's first pass (round >= 4, giving the normalize chain time);
   the last q-block's Wo runs at the tail with its PSUM->SBUF casts on
   ScalarE (idle there) instead of DVE.
 - softmax denominators via the ones-column of V_aug (row hd of avo);
   normalize: copy avo out of PSUM fast, reciprocal in a [128, 4]
   partition-scattered layout via a small SBUF DMA round-trip,
   partition_broadcast on GpSimd, multiply on DVE -> outT bf16.
 - PSUM: scps 2 bufs x [128,2,512]f32 (2 banks each) + avo pair
   (2 banks) + pjps 2 bufs x 1 bank for projection/Wo items = 8 banks.

Biases are structurally zero in this problem spec and are skipped.
"""

import numpy as np

HEADS = 16
D = 1024
N = 2048
B = 2
N_CORES = 8
HPC = HEADS // (N_CORES // B)  # heads per core = 4
HD = D // HEADS                # 64
F = HPC * HD                   # 256 features per core
P = 128


def build_nc(n=N, d=D, hpc=HPC, hd=HD):
    """Build the per-core Bass program (SPMD: same program on all 8 cores)."""
    import concourse.bass as bass
    import concourse.tile as tile
    from concourse import bacc, mybir

    f32 = mybir.dt.float32
    bf16 = mybir.dt.bfloat16
    f = hpc * hd            # per-core feature count (256)
    FC = f // P             # feature chunks / head pairs (2)
    DC = d // P             # contraction chunks over d (8)
    NT = n // P             # key chunks (16)
    QB = 512                # q-block width
    NQB = n // QB           # 4
    scale = 1.0 / float(np.sqrt(np.float32(d)))

    nc = bacc.Bacc("TRN2")

    xT = nc.declare_dram_parameter("xT", [d, n], bf16, isOutput=False)
    wqT = nc.declare_dram_parameter("wqT", [d, f], bf16, isOutput=False)
    wkT = nc.declare_dram_parameter("wkT", [d, f], bf16, isOutput=False)
    wvT = nc.declare_dram_parameter("wvT", [d, f], bf16, isOutput=False)
    woT = nc.declare_dram_parameter("woT", [f, d], bf16, isOutput=False)
    out = nc.declare_dram_parameter("out", [d, n], bf16, isOutput=True)

    xT_c = xT.rearrange("(c p) n -> c p n", p=P)
    wqT_c = wqT.rearrange("(c p) f -> c p f", p=P)
    wkT_c = wkT.rearrange("(c p) f -> c p f", p=P)
    wvT_c = wvT.rearrange("(c p) f -> c p f", p=P)
    woT_c = woT.rearrange("(c p) n -> c p n", p=P)

    with tile.TileContext(nc) as tc:
        with (
            tc.tile_pool(name="qkv", bufs=1) as qkv,
            tc.tile_pool(name="outT", bufs=1) as outp,
            tc.tile_pool(name="pt", bufs=2) as ptp,
            tc.tile_pool(name="norm", bufs=1) as normp,
            tc.tile_pool(name="xw", bufs=1) as xw,
            tc.tile_pool(name="wosb", bufs=4) as wosbp,
            tc.tile_pool(name="scps", bufs=2, space="PSUM") as scps,
            tc.tile_pool(name="avps", bufs=1, space="PSUM") as avps,
            tc.tile_pool(name="pjps", bufs=2, space="PSUM") as pjps,
        ):
            QT_sb = qkv.tile([P, FC, n], bf16)
            KT_sb = qkv.tile([P, FC, n], bf16)
            V_sb = qkv.tile([P, NT, hpc, hd + 1], bf16)
            outT_sb = outp.tile([P, FC, n], bf16)
            woT_sb = outp.tile([P, FC, d], bf16)
            # ones column of V_aug (accumulates softmax denominators in AV)
            ones_c = outp.tile([P, 1], bf16)
            nc.vector.memset(ones_c[:], 1.0)
            nc.vector.tensor_copy(
                V_sb[:, :, :, hd : hd + 1],
                ones_c.to_broadcast([P, NT, hpc, 1]),
            )

            xT_r = xw.tile([P, DC, n], bf16)
            wqT_r = xw.tile([P, DC, f], bf16)
            wkT_r = xw.tile([P, DC, f], bf16)
            wvT_r = xw.tile([P, DC, f], bf16)

            # wk + xT interleaved per chunk: the KT block-0 matmuls stream
            # right behind them; wq/wv arrive during KT/QT compute.
            for dc in range(DC):
                nc.sync.dma_start(out=wkT_r[:, dc, :], in_=wkT_c[dc])
                nc.sync.dma_start(out=xT_r[:, dc, :], in_=xT_c[dc])
                if dc == 0:
                    # ~3.4us of throwaway matmuls on the first wk chunk:
                    # trips the PE_HAM activity window during the DMA-paced
                    # phase so the lead-in projections run at 2.4 GHz
                    # instead of the cold 1.2 GHz default.
                    for w in range(16):
                        warm = pjps.tile([P, f], f32, tag="pj", name="warm")
                        nc.tensor.matmul(
                            warm[:],
                            wkT_r[:, 0, 0:P],
                            wkT_r[:, 0, :],
                            start=True,
                            stop=True,
                        )
            for dc in range(DC):
                nc.sync.dma_start(out=wqT_r[:, dc, :], in_=wqT_c[dc])
            for dc in range(DC):
                nc.sync.dma_start(out=wvT_r[:, dc, :], in_=wvT_c[dc])
            for fc in range(FC):
                nc.sync.dma_start(out=woT_sb[:, fc, :], in_=woT_c[fc])

            # ---- work items: each emits <=~900ns of PE work; projection
            # blocks are split into two 4-dc halves sharing one pjps tile ----
            half_state = {}

            def qk_half(w_sb, dest, fc, qc, half):
                key = (id(w_sb), fc, qc)
                sl = slice(qc * QB, (qc + 1) * QB)
                if half == 0:
                    ps = pjps.tile([P, QB], f32, tag="pj")
                    half_state[key] = ps
                else:
                    ps = half_state.pop(key)
                for dc in range(half * 4, half * 4 + 4):
                    nc.tensor.matmul(
                        ps[:],
                        w_sb[:, dc, fc * P : (fc + 1) * P],
                        xT_r[:, dc, sl],
                        start=(dc == 0),
                        stop=(dc == DC - 1),
                    )
                if half == 1:
                    nc.vector.tensor_copy(dest[:, fc, sl], ps[:])

            def v_half(nt, half):
                """V for key-chunk nt, all 4 heads (N=256 moving)."""
                key = ("v", nt)
                if half == 0:
                    ps = pjps.tile([P, QB], f32, tag="pj")
                    half_state[key] = ps
                else:
                    ps = half_state.pop(key)
                for dc in range(half * 4, half * 4 + 4):
                    nc.tensor.matmul(
                        ps[:, 0:f],
                        xT_r[:, dc, nt * P : (nt + 1) * P],
                        wvT_r[:, dc, :],
                        start=(dc == 0),
                        stop=(dc == DC - 1),
                    )
                if half == 1:
                    nc.vector.tensor_copy(
                        V_sb[:, nt, :, 0:hd],
                        ps[:, 0:f].rearrange("p (h e) -> p h e", h=hpc),
                    )

            def wo_item(qb, do, tail=False):
                """One do-chunk of the output projection for q-block qb
                (woT stationary; emits the partial TRANSPOSED [128, QB])."""
                q0 = qb * QB
                ps = pjps.tile([P, QB], f32, tag="pj")
                for fc in range(FC):
                    nc.tensor.matmul(
                        ps[:],
                        woT_sb[:, fc, do * P : (do + 1) * P],
                        outT_sb[:, fc, q0 : q0 + QB],
                        start=(fc == 0),
                        stop=(fc == FC - 1),
                    )
                ob = wosbp.tile([P, QB], bf16, tag="ob")
                if tail:
                    nc.scalar.copy(ob[:], ps[:])
                else:
                    nc.vector.tensor_copy(ob[:], ps[:])
                nc.sync.dma_start(
                    out=out[do * P : (do + 1) * P, q0 : q0 + QB],
                    in_=ob[:],
                )

            def pass_end(fc, qb, avos):
                """Free avo fast, then normalize rows 0..hd-1 by row hd (the
                softmax sums). reciprocal is single-lane-slow on a [1, QB]
                row, so scatter the sums across partitions via a small SBUF
                DMA round-trip first."""
                q0 = qb * QB
                for hi in range(2):
                    po = hi * hd
                    av_sb = normp.tile([hd + 1, QB], f32, tag=f"av_sb{hi}")
                    nc.vector.tensor_copy(av_sb[:], avos[hi][:])
                    rsh = normp.tile([P, QB // P], f32, tag=f"rsh{hi}")
                    nc.sync.dma_start(out=rsh[:], in_=av_sb[hd : hd + 1, :])
                    rsh2 = normp.tile([P, QB // P], f32, tag=f"rsh2{hi}")
                    nc.vector.reciprocal(rsh2[:], rsh[:])
                    recip = normp.tile([1, QB], f32, tag=f"recip{hi}")
                    nc.sync.dma_start(out=recip[:], in_=rsh2[:])
                    bc = normp.tile([hd, QB], f32, tag=f"bc{hi}")
                    nc.gpsimd.partition_broadcast(bc[:], recip[:])
                    nc.vector.tensor_mul(
                        outT_sb[po : po + hd, fc, q0 : q0 + QB],
                        av_sb[0:hd, :],
                        bc[:],
                    )

            def do_pass(fc, qb, fillers):
                """Attention pass for head pair fc (heads 2fc, 2fc+1) on
                q-block qb. fillers[r] = list of work items emitted between
                round r's scores and AV matmuls (they run in the exp
                shadow; the engine queue is FIFO so placement = pacing)."""
                q0 = qb * QB
                avos = [
                    avps.tile([hd + 1, QB], f32, tag=f"avo{i}", name=f"avo{i}")
                    for i in range(2)
                ]
                for kc in range(NT):
                    sc = scps.tile([P, 2, QB], f32, tag="sc")
                    for hi in range(2):
                        p0 = hi * hd
                        nc.tensor.matmul(
                            sc[:, hi, :],
                            KT_sb[p0 : p0 + hd, fc, kc * P : (kc + 1) * P],
                            QT_sb[p0 : p0 + hd, fc, q0 : q0 + QB],
                            start=True,
                            stop=True,
                        )
                    pt = ptp.tile([P, 2, QB], bf16, tag="pt")
                    nc.scalar.activation(
                        pt[:], sc[:], mybir.ActivationFunctionType.Exp,
                        scale=scale,
                    )
                    for item in fillers.get(kc, []):
                        item()
                    for hi in range(2):
                        nc.tensor.matmul(
                            avos[hi][:],
                            V_sb[:, kc, 2 * fc + hi, :],
                            pt[:, hi, :],
                            start=(kc == 0),
                            stop=(kc == NT - 1),
                        )
                pass_end(fc, qb, avos)

            # ---- emission schedule ----
            def QK(w, dest, fc, qc, half):
                return lambda: qk_half(w, dest, fc, qc, half)

            def VH(nt, half):
                return lambda: v_half(nt, half)

            def WO(qb, do, tail=False):
                return lambda: wo_item(qb, do, tail)

            # lead-in: minimal chain to the first exp (QT b0 before V so
            # exp(0) isn't gated behind the V tiles; AV(0) needs V(0) only
            # one exp later)
            qk_half(wkT_r, KT_sb, 0, 0, 0)
            qk_half(wkT_r, KT_sb, 0, 0, 1)
            qk_half(wqT_r, QT_sb, 0, 0, 0)
            qk_half(wqT_r, QT_sb, 0, 0, 1)
            v_half(0, 0)
            v_half(0, 1)
            v_half(1, 0)
            v_half(1, 1)

            # pass(0,0): carries V(2..15), KT fc0 b1-b3 (b_j needed by round
            # 4j), KT fc1 b0 and QT fc1 qb0 (needed by pass(1,0) round 0).
            f00 = {}
            for kc in range(14):
                f00[kc] = [VH(kc + 2, 0), VH(kc + 2, 1)]
            for i, it in enumerate(
                [QK(wkT_r, KT_sb, 0, 1, 0), QK(wkT_r, KT_sb, 0, 1, 1),
                 QK(wkT_r, KT_sb, 0, 2, 0), QK(wkT_r, KT_sb, 0, 2, 1),
                 QK(wkT_r, KT_sb, 0, 3, 0), QK(wkT_r, KT_sb, 0, 3, 1),
                 QK(wkT_r, KT_sb, 1, 0, 0), QK(wkT_r, KT_sb, 1, 0, 1),
                 QK(wqT_r, QT_sb, 1, 0, 0), QK(wqT_r, QT_sb, 1, 0, 1)]
            ):
                f00.setdefault(i + 2, []).append(it)
            do_pass(0, 0, f00)

            # pass(1,0): carries KT fc1 b1-b3 (b_j by round 4j) + QT qb1.
            f10 = {}
            for i, it in enumerate(
                [QK(wkT_r, KT_sb, 1, 1, 0), QK(wkT_r, KT_sb, 1, 1, 1),
                 QK(wkT_r, KT_sb, 1, 2, 0), QK(wkT_r, KT_sb, 1, 2, 1),
                 QK(wkT_r, KT_sb, 1, 3, 0), QK(wkT_r, KT_sb, 1, 3, 1),
                 QK(wqT_r, QT_sb, 0, 1, 0), QK(wqT_r, QT_sb, 0, 1, 1),
                 QK(wqT_r, QT_sb, 1, 1, 0), QK(wqT_r, QT_sb, 1, 1, 1)]
            ):
                f10.setdefault(i, []).append(it)
            do_pass(1, 0, f10)

            # q-blocks 1..3: wo(qb-1) rides in pass(0, qb) starting round 6
            # (the qb-1 normalize chain needs ~5us to land) on alternating
            # rounds, spilling into pass(1, qb); the next q-block's QT
            # blocks ride on pass(1, qb)'s later alternating rounds. One
            # item per busy round keeps the PE inside the exp shadow.
            for qb in range(1, NQB):
                fa = {}
                for i in range(5):
                    fa[6 + 2 * i] = [WO(qb - 1, i)]
                do_pass(0, qb, fa)
                fb = {}
                for i in range(3):
                    fb[2 * i] = [WO(qb - 1, 5 + i)]
                if qb < NQB - 1:
                    for i, it in enumerate(
                        [QK(wqT_r, QT_sb, 0, qb + 1, 0),
                         QK(wqT_r, QT_sb, 0, qb + 1, 1),
                         QK(wqT_r, QT_sb, 1, qb + 1, 0),
                         QK(wqT_r, QT_sb, 1, qb + 1, 1)]
                    ):
                        fb.setdefault(6 + 2 * i, []).append(it)
                do_pass(1, qb, fb)
            for do in range(d // P):
                wo_item(NQB - 1, do, tail=True)
    nc.finalize()
    return nc


def make_in_maps(x, Wq, Wk, Wv, Wo):
    """Shard full inputs into per-core DRAM parameter maps (bf16)."""
    import ml_dtypes

    bf = ml_dtypes.bfloat16
    x = np.asarray(x, dtype=np.float32)
    Wq = np.asarray(Wq, dtype=np.float32)
    Wk = np.asarray(Wk, dtype=np.float32)
    Wv = np.asarray(Wv, dtype=np.float32)
    Wo = np.asarray(Wo, dtype=np.float32)
    xTs = [np.ascontiguousarray(x[b].T).astype(bf) for b in range(B)]
    WqT, WkT, WvT = Wq.T, Wk.T, Wv.T
    in_maps = []
    for c in range(N_CORES):
        b, g = c // (N_CORES // B), c % (N_CORES // B)
        fs = slice(g * F, (g + 1) * F)
        in_maps.append(
            {
                "xT": xTs[b],
                "wqT": np.ascontiguousarray(WqT[:, fs]).astype(bf),
                "wkT": np.ascontiguousarray(WkT[:, fs]).astype(bf),
                "wvT": np.ascontiguousarray(WvT[:, fs]).astype(bf),
                "woT": np.ascontiguousarray(Wo[:, fs].T).astype(bf),
            }
        )
    return in_maps


_NC_CACHE = {}


def run(x, Wq, Wk, Wv, Wo, trace=False, **kw):
    from concourse.bass_utils import run_bass_kernel_spmd

    if "nc" not in _NC_CACHE:
        _NC_CACHE["nc"] = build_nc()
    nc = _NC_CACHE["nc"]
    in_maps = make_in_maps(x, Wq, Wk, Wv, Wo)
    res = run_bass_kernel_spmd(
        nc, in_maps, core_ids=list(range(N_CORES)), trace=trace, **kw
    )
    parts = [
        np.asarray(res.results[i]["out"]).astype(np.float32)
        for i in range(N_CORES)
    ]
    gpb = N_CORES // B
    # per-core partials are transposed [d, n]: sum the group, then untranspose
    full = np.stack(
        [
            sum(parts[b * gpb + 1 : (b + 1) * gpb], parts[b * gpb]).T
            for b in range(B)
        ]
    )
    return np.ascontiguousarray(full, dtype=np.float32), res


def kernel(x, Wq, bq, Wk, bk, Wv, bv, Wo, bo):
    full, _ = run(x, Wq, Wk, Wv, Wo)
    return full
